# revision 20
# baseline (speedup 1.0000x reference)
"""FAME-GCN Trainium2 kernel (bf16 redesign).

Computes, for merged adjacency final_A = sum_k w_k A_k + (sum_k w_k A_k)^T:
    U1 = final_A @ (feature @ W3) + b3
    U2 = final_A2 @ (feature @ W1) + b1
    out = concat(U1, U2, axis=1)          # [5000, 32]

Distribution: node rows sharded 625/core across 8 NeuronCores; the
[N, 16] transpose-side partials are summed across cores on the host
(the reduce-scatter of the row-sharded spmm); biases + concat on host.

Per core (vs the f32 baseline, ~2x less HBM traffic and ~4x less PE
streaming):
  - relations are pre-scaled by their merge weight on the host and cast
    to bf16, so the device merge is plain adds and all PE work is bf16,
  - 12 full-width relation stripes [125, 5120] are gathered per stripe
    on 4 SWDGE queues (the only DGE path that spreads this pattern over
    all 16 SDMA engines),
  - merge = in-place bf16 tensor_adds (t0 += t1 += ... ; t3 += t4...),
  - each chunk is pushed through ONE normal matmul with the merged
    chunk stationary and rhs = [I_126 | S_own] (142 cols): psum cols
    0:126 are the transposed chunk, cols 126:142 the dir1 product
    (T^T S chunk) — no separate dir1 matmuls or transposes; per-stripe
    bf16 dir1 partials are extracted and summed on the host. Three
    chunks share a psum bank (first with start=True to clear it),
  - dir2 (row side, (T S)[r, m]): ACT copies each 3-chunk strip
    [128, 3, 142] to SBUF as bf16; flipped matmuls chain the 40
    transposed chunks into a [126, 16] psum accumulator per
    (stripe, group).
"""

import sys

if "/opt/trn_rl_repo" not in sys.path:
    sys.path.insert(0, "/opt/trn_rl_repo")

import numpy as np
import ml_dtypes

import concourse.bacc as bacc
import concourse.mybir as mybir
from concourse.tile import TileContext
from concourse.bass_utils import run_bass_kernel_spmd

BF16 = mybir.dt.bfloat16
F32 = mybir.dt.float32

N = 5000
NP = 5120  # padded row length in dram
NFEAT = 128
OUT = 16
K_A, K_AT = 3, 9
KT = K_A + K_AT  # 12
NCORES = 8
RS = N // NCORES  # 625
STRIPE = 125
NSTRIPE = RS // STRIPE  # 5
NCHUNK = (N + 127) // 128  # 40 (last chunk 8 wide)
NSTRIP = NCHUNK // 4  # 10 strips of 4 chunks
TW = 126  # transposed chunk width in the strip tile

_CACHE = {}


def _cjw(j):
    return min(128, N - j * 128)


def build():
    nc = bacc.Bacc(num_swdge_queues=4)

    adj = nc.declare_dram_parameter("adj", [KT * RS, NP], BF16, isOutput=False)
    idxs = nc.declare_dram_parameter(
        "idxs", [128, 16 * 6 * NSTRIPE], mybir.dt.int16, isOutput=False
    )
    featT = nc.declare_dram_parameter("featT", [NFEAT, NP], BF16, isOutput=False)
    featow = nc.declare_dram_parameter("featow", [NFEAT, 640], BF16, isOutput=False)
    wcat = nc.declare_dram_parameter("wcat", [NFEAT, 2 * OUT], BF16, isOutput=False)
    ident = nc.declare_dram_parameter("ident", [128, 128], BF16, isOutput=False)

    # dir1 per-stripe partials: slot (g, j=4s+q) at [st, :, g*40+j, :]
    o1x = nc.declare_dram_parameter(
        "o1x", [NSTRIPE, 128, 2 * NCHUNK, OUT], BF16, isOutput=True
    )
    # dir2 rows: group g at cols [g*80, (g+1)*80), stripe st at st*16
    o2 = nc.declare_dram_parameter("o2", [128, 2 * 80], F32, isOutput=True)

    with TileContext(nc) as tc:
        with (
            tc.tile_pool(name="persist", bufs=1) as pp,
        ):
            wct = pp.tile([NFEAT, 2 * OUT], BF16, tag="wct")
            idt = pp.tile([128, 128], BF16, tag="idt")
            nc.sync.dma_start(out=wct, in_=wcat[:, :])
            nc.sync.dma_start(out=idt, in_=ident[:, :])
            ix = pp.tile([128, 16 * 6 * NSTRIPE], mybir.dt.int16, tag="ix")
            nc.sync.dma_start(out=ix, in_=idxs[:, :])

            s3f = pp.tile([128, NCHUNK * OUT], BF16, tag="s3f")
            s1f = pp.tile([128, NCHUNK * OUT], BF16, tag="s1f")
            acc2 = pp.tile([128, 2 * 80], F32, tag="acc2")
            # transpose rhs per (g, st): [I_126 | S_own_g_st] at cols v*XW
            xid = pp.tile([128, 10 * (TW + OUT)], BF16, tag="xid")

            # ---------------- preamble: support matrices ----------------
            with (
                tc.tile_pool(name="prep", bufs=1) as prp,
                tc.tile_pool(name="pmS", bufs=4, space="PSUM") as pms,
            ):
                ftT = prp.tile([NFEAT, N], BF16, tag="ftT")
                nc.sync.dma_start(out=ftT, in_=featT[:, 0:N])
                fow = prp.tile([NFEAT, 640], BF16, tag="fow")
                nc.sync.dma_start(out=fow, in_=featow[:, :])
                for t in range(NCHUNK):
                    wt = _cjw(t)
                    ps = pms.tile([128, 2 * OUT], F32, tag="pmS",
                                  name=f"ps_{t}")
                    nc.tensor.matmul(
                        ps[:wt, :], ftT[:, t * 128 : t * 128 + wt], wct,
                        start=True, stop=True,
                    )
                    nc.scalar.copy(
                        out=s3f[:wt, t * OUT : (t + 1) * OUT], in_=ps[:wt, 0:OUT]
                    )
                    nc.scalar.copy(
                        out=s1f[:wt, t * OUT : (t + 1) * OUT],
                        in_=ps[:wt, OUT : 2 * OUT],
                    )
                XW = TW + OUT  # 142
                for v in range(10):
                    nc.vector.tensor_copy(
                        out=xid[:, v * XW : v * XW + TW], in_=idt[:, 0:TW]
                    )
                for st in range(NSTRIPE):
                    ps = pms.tile([128, 2 * OUT], F32, tag="pmS",
                                  name=f"pso_{st}")
                    nc.tensor.matmul(
                        ps[:STRIPE, :],
                        fow[:, st * STRIPE : (st + 1) * STRIPE],
                        wct, start=True, stop=True,
                    )
                    for g in range(2):
                        v = g * NSTRIPE + st
                        nc.scalar.copy(
                            out=xid[:STRIPE, v * XW + TW : (v + 1) * XW],
                            in_=ps[:STRIPE, g * OUT : (g + 1) * OUT],
                        )

            # ---------------- main loop: stripe-major ----------------
            XW = TW + OUT  # 142
            with (
                tc.tile_pool(name="raw", bufs=9) as rawp,
                tc.tile_pool(name="stp", bufs=4) as stp,
                tc.tile_pool(name="oxp", bufs=2) as oxp,
                tc.tile_pool(name="ptp", bufs=4, space="PSUM") as ptp,
                tc.tile_pool(name="pd2p", bufs=2, space="PSUM") as pd2p,
            ):
                for st in range(NSTRIPE):
                    # 6 pair-gathers per stripe: relations (2p, 2p+1) stacked
                    # in one SWDGE gather of 256 row indices.  Group-b pairs
                    # (P2..P5, P1) are issued first so the long merge chain
                    # and b's PE work start as early as possible; group-a's
                    # P0 streams last.
                    pairs = [None] * 6
                    for i, p6 in enumerate((2, 3, 4, 5, 1, 0)):
                        tk = rawp.tile([128, 2, NP], BF16, tag="raw",
                                       name=f"t_{st}_{p6}")
                        ixs = ix[:, (st * 6 + p6) * 16 : (st * 6 + p6 + 1) * 16]
                        nc.gpsimd.dma_gather(
                            tk,
                            adj[:, :],
                            ixs,
                            256,
                            256,
                            NP,
                            elem_step=NP,
                            queue_num=(st * 6 + i) % 4,
                        )
                        pairs[p6] = tk

                    def rel(k):
                        return pairs[k // 2][:, k % 2, 0:N]

                    # merge: in-place bf16 adds on DVE, ordered by arrival.
                    # group b (k=3..11) accumulates into rel(4) (arrives
                    # first); group a (k=0,1,2) into rel(0)
                    for k in (5, 6, 7, 8, 9, 10, 11, 3):
                        nc.vector.tensor_add(rel(4), rel(4), rel(k))
                    nc.vector.tensor_add(rel(0), rel(0), rel(2))
                    nc.vector.tensor_add(rel(0), rel(0), rel(1))

                    ox = oxp.tile([128, 2 * NCHUNK, OUT], BF16, tag="ox",
                                  name=f"ox_{st}")
                    NS3 = (NCHUNK + 2) // 3  # 14 strips of <=3 chunks
                    for g, mrg, sf in (
                        (1, rel(4), s1f), (0, rel(0), s3f)
                    ):
                        v = g * NSTRIPE + st
                        xslice = xid[:STRIPE, v * XW : (v + 1) * XW]
                        pd2 = pd2p.tile([128, OUT], F32, tag="pd2",
                                        name=f"pd2_{st}_{g}")
                        for s in range(NS3):
                            js = list(range(3 * s, min(3 * s + 3, NCHUNK)))
                            # fused op: [chunk^T | chunk^T @ S_own] per chunk;
                            # first chunk start=True clears the psum bank
                            pt = ptp.tile([128, 3, XW], F32, tag="pt",
                                          name=f"pt_{st}_{g}_{s}")
                            for qi, j in enumerate(js):
                                cjw = _cjw(j)
                                nc.tensor.matmul(
                                    pt[:cjw, qi, :],
                                    mrg[:STRIPE, j * 128 : j * 128 + cjw],
                                    xslice,
                                    start=(qi == 0),
                                    stop=(qi == len(js) - 1),
                                )
                            strip = stp.tile([128, 3, XW], BF16, tag="strip",
                                             name=f"strip_{st}_{g}_{s}")
                            nc.scalar.copy(out=strip, in_=pt)
                            # dir1 partials out (psum cols 126:142 per chunk)
                            nc.vector.tensor_copy(
                                out=ox[:, g * NCHUNK + 3 * s
                                       : g * NCHUNK + 3 * s + len(js), :],
                                in_=strip[:, 0 : len(js), TW:XW],
                            )
                            for qi, j in enumerate(js):
                                cjw = _cjw(j)
                                nc.tensor.matmul(
                                    pd2[:TW, :],
                                    strip[:cjw, qi, 0:TW],
                                    sf[:cjw, j * OUT : (j + 1) * OUT],
                                    start=(j == 0),
                                    stop=(j == NCHUNK - 1),
                                )
                        nc.vector.tensor_copy(
                            out=acc2[:STRIPE,
                                     g * 80 + st * OUT : g * 80 + (st + 1) * OUT],
                            in_=pd2[:STRIPE, :],
                        )
                    nc.sync.dma_start(out=o1x[st, :, :, :], in_=ox)

            nc.sync.dma_start(out=o2[:, :], in_=acc2)

    nc.compile()
    return nc


def _make_inputs(feature, A, A_t, w2, wb, W3, W1):
    bf = ml_dtypes.bfloat16
    featT_full = np.zeros((NFEAT, NP), dtype=bf)
    featT_full[:, :N] = feature.T.astype(bf)
    ident = np.eye(128, dtype=np.float32).astype(bf)
    wcat = np.concatenate([W3, W1], axis=1).astype(bf)

    wall = np.concatenate([w2, wb]).astype(np.float32)  # [12]

    # gather indices for 2-relation stacked gathers: gather slot
    # (st, p6) has 256 indices; q<128 -> relation 2*p6 row st*125+(q),
    # q>=128 -> relation 2*p6+1 (q-128).  idx q lives at
    # [q%16 + 16*rep, 16*slot + q//16], -1 entries are skipped.
    idxs = np.full((128, 16 * 6 * NSTRIPE), -1, dtype=np.int16)
    for st in range(NSTRIPE):
        for p6 in range(6):
            slot = st * 6 + p6
            for q in range(256):
                k = 2 * p6 + q // 128
                j = q % 128
                if j >= STRIPE:
                    continue
                val = k * RS + st * STRIPE + j
                for rep in range(8):
                    idxs[q % 16 + 16 * rep, 16 * slot + q // 16] = val

    in_maps = []
    for p in range(NCORES):
        r0 = p * RS
        adj = np.empty((KT * RS, NP), dtype=bf)
        for k in range(KT):
            src = A[k] if k < K_A else A_t[k - K_A]
            adj[k * RS : (k + 1) * RS, :N] = (
                src[r0 : r0 + RS] * wall[k]
            ).astype(bf)
            adj[k * RS : (k + 1) * RS, N:] = 0
        featow = np.zeros((NFEAT, 640), dtype=bf)
        featow[:, :RS] = feature[r0 : r0 + RS].T.astype(bf)
        in_maps.append(
            {
                "adj": adj,
                "idxs": idxs,
                "featT": featT_full,
                "featow": featow,
                "wcat": wcat,
                "ident": ident,
            }
        )
    return in_maps


def kernel(feature, A, A_t, weight_b2, weight_b, W3, b3, W1, b1, **kw):
    feature = np.asarray(feature, dtype=np.float32)
    A = np.asarray(A, dtype=np.float32)
    A_t = np.asarray(A_t, dtype=np.float32)
    w2 = np.asarray(weight_b2, dtype=np.float32).reshape(K_A)
    wb = np.asarray(weight_b, dtype=np.float32).reshape(K_AT)
    W3 = np.asarray(W3, dtype=np.float32)
    W1 = np.asarray(W1, dtype=np.float32)
    b3 = np.asarray(b3, dtype=np.float32)
    b1 = np.asarray(b1, dtype=np.float32)

    if "nc" not in _CACHE:
        _CACHE["nc"] = build()
    nc = _CACHE["nc"]

    in_maps = _make_inputs(feature, A, A_t, w2, wb, W3, W1)
    _CACHE["in_maps"] = in_maps

    res = run_bass_kernel_spmd(nc, in_maps, core_ids=list(range(NCORES)))

    # dir1: sum the per-stripe bf16 partials over (stripes, cores);
    # slot (g, j) at [:, g*40+j, :]
    o1sum = np.zeros((128, 2 * NCHUNK, OUT), dtype=np.float32)
    for p in range(NCORES):
        o1sum += res.results[p]["o1x"].astype(np.float32).sum(axis=0)

    cols = np.empty((2, N, OUT), dtype=np.float32)
    rows = np.empty((2, N, OUT), dtype=np.float32)
    for g in range(2):
        for j in range(NCHUNK):
            cjw = _cjw(j)
            cols[g, j * 128 : j * 128 + cjw] = o1sum[:cjw, g * NCHUNK + j]
    for p in range(NCORES):
        o2p = res.results[p]["o2"]
        for g in range(2):
            for st in range(NSTRIPE):
                r = p * RS + st * STRIPE
                rows[g, r : r + STRIPE] = o2p[
                    :STRIPE, g * 80 + st * OUT : g * 80 + (st + 1) * OUT
                ]

    U1 = cols[0] + rows[0] + b3
    U2 = cols[1] + rows[1] + b1
    return np.concatenate([U1, U2], axis=1).astype(np.float32)
